# revision 19
# baseline (speedup 1.0000x reference)
"""Trainium2 Bass kernel for nn_DispersiveLoss (B=2048, D=16*768=12288, 8 cores).

Single-launch circulant block decomposition (uniform SPMD, no collectives):
  x (2048, 12288) -> 16 row-blocks of 128. Core c owns m-blocks {2c, 2c+1}
  and computes two Gram strips G[m, m..m+8 (mod 16)] (width 9 blocks = 1152)
  in fp8 DoubleRow (D on partitions, 48 double-k-chunks, PSUM f32 accum).
  The PE does ONLY this k-loop.

  Row norms come for free: sq_p = diag(G[m,m]) extracted from PSUM with an
  identity mask at k-loop end. The row-term correction u_p = -(sq_p - D)/2
  folds into the ACT Exp bias (per partition, local). The column-term
  correction u_j needs remote rows' norms, so it is NOT applied on device;
  instead the device exports, per strip:
    - CE_j = sum_p exp(2SS*g_pj + 2SS*u_p)   (GPSIMD cross-partition reduce)
    - CG_j = sum_p g_pj                      (GPSIMD cross-partition reduce)
    - per-partition region sums of g and g^2 (S1*, S2*), and u_p
  and the host (float64, O(B^2/128) linear work) applies the exact
  region-weighted combination:
    E  = sum w_j a_j CE_j - 1024 e^4,  a_j = exp(2SS u_j)
    T1 = sum S1c + 128 sum w_j u_j + 1024 Uown - 1024 D
    T2 = sum S2c + 2 sum w_j u_j CG_j + 128 sum w_j u_j^2
         + 2 sum_p u_p S1c_p + 1024 Usq + 2 Uown sum w_j u_j - 1024 D^2
  with column weights w = [0.5]*128 + [1]*896 + [0.5]*128 (diag block and
  the twice-computed distance-8 block at half weight; self-pairs are the
  exact constants removed by the 1024-terms since p_ii = D identically).
"""

import os

import numpy as np
import ml_dtypes

import concourse.bass as bass
import concourse.mybir as mybir
import concourse.tile as tile
from concourse import bacc
from concourse.bass_utils import run_bass_kernel_spmd

NC_N = 8
B, D = 2048, 12288
BLK = 128
UNION = 1280  # 10 blocks per core in SBUF
STRIPW = 1152  # 9-block strip width
KCH = 96  # k-chunks of 128
KB = 4  # k-chunks per DMA slab
TAU = 0.5
CC = float(2 * D)  # centering constant (E[d2] for N(0,1) rows)
SS = 1.0 / (D * TAU)  # exponent scale
S2E = 2.0 * SS
F32 = mybir.dt.float32
BF16 = mybir.dt.bfloat16
DT_IN = mybir.dt.float8e4
NP_IN = ml_dtypes.float8_e4m3

N_PAIRS = B * (B - 1) // 2

KERNEL_EXEC_NS = []  # filled when KERNEL_TRACE is set (test harness only)

_cache = {}


def _trace_enabled():
    return bool(os.environ.get("KERNEL_TRACE"))


def _build_kernel():
    nc = bacc.Bacc("TRN2", target_bir_lowering=False, debug=False, num_devices=NC_N)
    xT = nc.dram_tensor("xT", [BLK, KCH, UNION], DT_IN, kind="ExternalInput")
    ident = nc.dram_tensor("ident", [BLK, BLK], F32, kind="ExternalInput")
    out_stats = nc.dram_tensor("out_stats", [BLK, 14], F32, kind="ExternalOutput")
    out_cols = nc.dram_tensor("out_cols", [4, STRIPW], F32, kind="ExternalOutput")

    MULT = mybir.AluOpType.mult
    ADD = mybir.AluOpType.add
    EXP = mybir.ActivationFunctionType.Exp
    DR = mybir.MatmulPerfMode.DoubleRow
    X = mybir.AxisListType.X
    CAX = mybir.AxisListType.C

    segs = [(0, 512), (512, 1024), (1024, 1152)]

    # out_stats cols: 0,1 u (strip 0/1); per strip s at 2+6s:
    #   S1w S1d S1h S2w S2d S2h
    # out_cols rows: CE0, CG0, CE1, CG1
    with tile.TileContext(nc) as tc:
        with (
            tc.tile_pool(name="slab", bufs=8) as slab_pool,
            tc.tile_pool(name="psp", bufs=1, space="PSUM") as psp,
            tc.tile_pool(name="post", bufs=2) as post,
            tc.tile_pool(name="accp", bufs=1) as accp,
        ):
            ps0 = psp.tile([BLK, STRIPW], F32, tag="ps0")
            ps1 = psp.tile([BLK, STRIPW], F32, tag="ps1")
            ps = [ps0, ps1]
            acc = accp.tile([BLK, 14], F32)
            colsb = accp.tile([1, 4 * STRIPW], F32)

            # --- PE pre-warm: trip the HAM busy window while slab 0 DMAs ---
            warm = post.tile([BLK, 512], DT_IN, tag="warm")
            nc.gpsimd.memset(warm[:], 0.0)
            wps = psp.tile([BLK, 512], F32, tag="wps")
            for _ in range(8):
                nc.tensor.matmul(
                    wps[:], warm[:, 0:128], warm[:], start=True, stop=True,
                    skip_group_check=True,
                )
            ones128 = accp.tile([BLK, 1], BF16)
            nc.gpsimd.memset(ones128[:], 1.0)
            cps2 = psp.tile([1, 512], F32, tag="cps2")
            idt = accp.tile([BLK, BLK], F32)
            nc.sync.dma_start(idt[:], ident[:])
            # preload the Exp activation table early (the only table used)
            zcol = accp.tile([BLK, 1], F32)
            nc.gpsimd.memset(zcol[:], 0.0)
            zscr = accp.tile([BLK, 1], F32)
            nc.scalar.activation(zscr[:], zcol[:], EXP)

            SLAB_KS = [2, 2] + [4] * 23  # k-chunks per slab (first slabs small)
            NSLAB = len(SLAB_KS)
            OFFS = [sum(SLAB_KS[:i]) for i in range(NSLAB)]
            sts = []
            for kb in range(NSLAB):
                kk = SLAB_KS[kb]
                o = OFFS[kb]
                st = slab_pool.tile([BLK, kk, UNION], DT_IN, name="st", tag=f"slab{kk}")
                sts.append(st)
                nc.sync.dma_start(st[:], xT[:, o : o + kk, :])

                if kb < NSLAB - 8:
                    for kp in range(kk // 2):
                        ii = 2 * kp
                        for s in range(2):
                            off = 128 * s
                            lhs = st[:, ii : ii + 2, off : off + 128]
                            for c0, c1 in segs:
                                nc.tensor.matmul(
                                    ps[s][:, c0:c1],
                                    lhs,
                                    st[:, ii : ii + 2, off + c0 : off + c1],
                                    start=(kb == 0 and kp == 0),
                                    stop=False,
                                    perf_mode=DR,
                                )

            def strip_tail_mms(s):
                off = 128 * s
                for kb in range(NSLAB - 8, NSLAB):
                    st = sts[kb]
                    kk = SLAB_KS[kb]
                    last = kb == NSLAB - 1
                    for kp in range(kk // 2):
                        ii = 2 * kp
                        lhs = st[:, ii : ii + 2, off : off + 128]
                        for j, (c0, c1) in enumerate(segs):
                            nc.tensor.matmul(
                                ps[s][:, c0:c1],
                                lhs,
                                st[:, ii : ii + 2, off + c0 : off + c1],
                                start=False,
                                stop=(last and kp == kk // 2 - 1 and j == len(segs) - 1),
                                perf_mode=DR,
                            )

            e_t = [None, None]
            g16_t = [None, None]

            def strip_post_compute(s):
                p = ps[s]
                base = 2 + 6 * s
                # sq_p of own m-block from the Gram diagonal
                dscr = post.tile([BLK, BLK], F32, tag="dscr")
                dcol = accp.tile([BLK, 1], F32, tag=f"d{s}")
                nc.vector.scalar_tensor_tensor(
                    out=dscr[:], in0=p[:, 0:128], scalar=1.0, in1=idt[:],
                    op0=MULT, op1=MULT, accum_out=dcol[:],
                )
                # u_p = -(sq - D)/2 (exported); bias = 2SS*u_p = -SS*(sq - D)
                nc.vector.tensor_scalar(
                    out=acc[:, s : s + 1], in0=dcol[:], scalar1=-float(D),
                    scalar2=-0.5, op0=ADD, op1=MULT,
                )
                bias = accp.tile([BLK, 1], F32, tag=f"b{s}")
                nc.vector.tensor_scalar(
                    out=bias[:], in0=dcol[:], scalar1=-SS, scalar2=SS * float(D),
                    op0=MULT, op1=ADD,
                )
                # e = exp(2SS*g + 2SS*u_p); g16 = bf16 copy of g with S1w accum
                e = post.tile([BLK, STRIPW], BF16, tag="e")
                e_t[s] = e
                nc.scalar.activation(e[:], p[:, 0:STRIPW], EXP, bias=bias[:], scale=S2E)
                g16 = post.tile([BLK, STRIPW], BF16, tag="g16")
                g16_t[s] = g16
                nc.vector.tensor_scalar(
                    out=g16[:], in0=p[:, 0:STRIPW], scalar1=1.0, scalar2=0.0,
                    op0=MULT, op1=ADD, accum_out=acc[:, base : base + 1],
                )
                # S2w from g16*g (one wide DVE op)
                scrd = post.tile([BLK, STRIPW], F32, tag="scrd")
                nc.vector.scalar_tensor_tensor(
                    out=scrd[:], in0=g16[:], scalar=1.0, in1=p[:, 0:STRIPW],
                    op0=MULT, op1=MULT, accum_out=acc[:, base + 3 : base + 4],
                )
                # narrow D/H region sums straight off PSUM
                nd = post.tile([BLK, BLK], F32, tag="nd")
                nc.vector.tensor_scalar(
                    out=nd[:], in0=p[:, 0:128], scalar1=1.0, scalar2=0.0,
                    op0=MULT, op1=ADD, accum_out=acc[:, base + 1 : base + 2],
                )
                s2d = post.tile([BLK, BLK], F32, tag="s2d")
                nc.vector.scalar_tensor_tensor(
                    out=s2d[:], in0=nd[:], scalar=1.0, in1=p[:, 0:128],
                    op0=MULT, op1=MULT, accum_out=acc[:, base + 4 : base + 5],
                )
                nh = post.tile([BLK, BLK], F32, tag="nh")
                nc.vector.tensor_scalar(
                    out=nh[:], in0=p[:, 1024:1152], scalar1=1.0, scalar2=0.0,
                    op0=MULT, op1=ADD, accum_out=acc[:, base + 2 : base + 3],
                )
                s2h = post.tile([BLK, BLK], F32, tag="s2h")
                nc.vector.scalar_tensor_tensor(
                    out=s2h[:], in0=nh[:], scalar=1.0, in1=p[:, 1024:1152],
                    op0=MULT, op1=MULT, accum_out=acc[:, base + 5 : base + 6],
                )

            def strip_colsums(s):
                # column sums via PE ones-matmuls (bf16 rhs), staged
                # through two PSUM banks; copy-out split across DVE and ACT
                if s == 0:
                    banks = [wps[0:1, 0:512], cps2[:, 0:512]]
                else:
                    banks = [
                        wps[0:1, 0:512], cps2[:, 0:512],
                        ps0[0:1, 0:512], ps0[0:1, 512:1024],
                    ]
                nb = len(banks)
                for mi, mat in enumerate((e_t[s], g16_t[s])):
                    row = 2 * s + mi
                    for j, (c0, c1) in enumerate(segs):
                        k = mi * 3 + j
                        tgt = banks[k % nb][:, 0 : c1 - c0]
                        nc.tensor.matmul(
                            tgt, ones128[:], mat[:, c0:c1],
                            start=True, stop=True, skip_group_check=True,
                        )
                        if k % 2 == 0:
                            nc.vector.tensor_copy(
                                colsb[:, row * STRIPW + c0 : row * STRIPW + c1], tgt
                            )
                        else:
                            nc.scalar.activation(
                                colsb[:, row * STRIPW + c0 : row * STRIPW + c1],
                                tgt, mybir.ActivationFunctionType.Copy,
                            )

            strip_tail_mms(0)
            strip_post_compute(0)
            strip_tail_mms(1)
            strip_colsums(0)
            strip_post_compute(1)
            strip_colsums(1)

            nc.sync.dma_start(out_stats[:], acc[:])
            nc.sync.dma_start(
                out_cols[:].rearrange("a b -> (a b)").rearrange("(a b) -> a b", a=1),
                colsb[:],
            )
    nc.compile()
    return nc


def _get(name, builder):
    if name not in _cache:
        _cache[name] = builder()
    return _cache[name]


def _run(nc, in_maps, tag):
    if _trace_enabled():
        try:
            import profhook

            profhook.install()
        except Exception:
            pass
        import tempfile

        res = run_bass_kernel_spmd(
            nc, in_maps, list(range(NC_N)), trace=True,
            tmpdir=tempfile.mkdtemp(prefix=f"ktrace_{tag}_"),
        )
        KERNEL_EXEC_NS.append((tag, res.exec_time_ns))
        return res.results
    return run_bass_kernel_spmd(nc, in_maps, list(range(NC_N))).results


def kernel(features):
    x = np.asarray(features).reshape(B, D)
    xbf = x.astype(NP_IN)

    xT_full = np.ascontiguousarray(xbf.T)  # (D, B)
    ident = np.eye(BLK, dtype=np.float32)
    in_maps = []
    for c in range(NC_N):
        cols = (256 * c + np.arange(UNION)) % B
        xu = xT_full[:, cols].reshape(KCH, BLK, UNION).transpose(1, 0, 2)
        in_maps.append({"xT": np.ascontiguousarray(xu), "ident": ident})
    nc_k = _get("main", _build_kernel)
    res = _run(nc_k, in_maps, "main")

    # ---- host combine (float64 linear combination of device partials) ----
    Dv = float(D)
    # global u vector from per-core exports
    u = np.zeros(B, dtype=np.float64)
    for c in range(NC_N):
        o = res[c]["out_stats"].astype(np.float64)
        u[256 * c : 256 * c + 128] = o[:, 0]
        u[256 * c + 128 : 256 * c + 256] = o[:, 1]
    a = np.exp(S2E * u)
    w = np.concatenate(
        [np.full(128, 0.5), np.full(896, 1.0), np.full(128, 0.5)]
    )

    E = T1 = T2 = 0.0
    for c in range(NC_N):
        o = res[c]["out_stats"].astype(np.float64)
        cols = res[c]["out_cols"].astype(np.float64)
        for s in range(2):
            base = 2 + 6 * s
            u_p = o[:, s]
            S1c = o[:, base] - 0.5 * (o[:, base + 1] + o[:, base + 2])
            S2c = o[:, base + 3] - 0.5 * (o[:, base + 4] + o[:, base + 5])
            CE = cols[2 * s]
            CG = cols[2 * s + 1]
            gidx = (256 * c + 128 * s + np.arange(STRIPW)) % B
            uj = u[gidx]
            aj = a[gidx]
            wu = w * uj
            E += np.dot(w * aj, CE)
            T1 += S1c.sum() + 128.0 * wu.sum() + 1024.0 * u_p.sum()
            T2 += (
                S2c.sum()
                + 2.0 * np.dot(wu, CG)
                + 128.0 * np.dot(wu, uj)
                + 2.0 * np.dot(u_p, S1c)
                + 1024.0 * np.dot(u_p, u_p)
                + 2.0 * u_p.sum() * wu.sum()
            )
    E -= 1024.0 * np.exp(S2E * Dv)
    T1 -= 1024.0 * Dv
    T2 -= 1024.0 * Dv * Dv

    sq = Dv - 2.0 * u
    feat_norm = np.sqrt(sq).sum() / B

    N = float(N_PAIRS)
    mean_u = -2.0 * T1 / N
    mean = (mean_u + CC) / D
    var_u = (4.0 * T2 - N * mean_u * mean_u) / (N - 1.0)
    std = np.sqrt(var_u) / D
    loss = CC * SS - np.log(E) + np.log(N)

    return (
        np.float32(loss),
        np.float32(feat_norm),
        np.float32(mean),
        np.float32(std),
    )


if __name__ == "__main__":
    f = np.random.default_rng(0).standard_normal((B, 16, 768), dtype=np.float32)
    print(kernel(features=f))
